# revision 13
# baseline (speedup 1.0000x reference)
"""GQA attention kernel for 8 trn2 NeuronCores.

Sharding: core = (b, h) with b = core//4 (batch), h = core%4 (kv head).
Each core handles q heads 4h..4h+3 (a contiguous 512-column block of Wq),
its own kv head (128 rows of Wk/Wv), and the matching 512-column slice of
Wo.  Per-core output is a partial y (row-parallel Wo); host sums the 4
fp16 partials per batch in fp32.

All matmuls run in fp16 (full-rate at 2.4 GHz) with fp32 PSUM
accumulation.  The attention j-loop is exp-bound on the scalar engine,
so the q projection of head g+1 is software-pipelined into head g's
attention loop (one projection matmul per j iteration) to keep the PE
busy during exp waits.  Softmax normalization: an all-ones [128,128]
matmul gives the partition-broadcast key sum in one PE op, followed by
a full-width DVE approx reciprocal and multiply.

DMA priority order feeds the pipeline, with block 0 of x split into two
separate 256-column tiles so the first k-projection never waits on
bytes it does not read (tile-granular DMA deps).  A redundant full
512-wide copy of block 0 (arriving late, off the critical path) serves
the pipelined q projections, which need 512-wide rhs slices.

Stationary-weight changes cost ~17ns/matmul on the PE queue, so loops
are ordered to reuse lhsT: q projections step two n-blocks per e-chunk
(one weight load feeds two matmuls), the in-attention output projection
(cfill) runs g-outer over ob pairs, and the tail output projection
accumulates all four 512-col blocks of one n-tile in a 4-bank PSUM
tile so each OT head slice is loaded once per tile.
"""

import numpy as np

EMB = 2048
N = 2048          # sequence length
HD = 128          # head dim
NHC = 4           # q heads per core
DQ = NHC * HD     # 512: per-core q concat dim
EC = 16           # e chunks of 128
SC = 16           # s chunks of 128
NB = 512          # n block size
NQ = 4            # n quarters in attention phase
SCALE = 1.0 / np.sqrt(HD)

_NC = None


def _build():
    import concourse.bass as bass
    from concourse import bacc
    import concourse.mybir as mybir
    import concourse.tile as tile
    from concourse.bass import ts

    FP32 = mybir.dt.float32
    F16 = mybir.dt.float16
    P = 128

    nc = bacc.Bacc("TRN2", target_bir_lowering=False, debug=False, num_devices=8)
    # all inputs host-prearranged to (partition, chunk, col) so every DMA
    # reads long contiguous runs (full-rate instead of 256-512B descriptors)
    x0a_d = nc.declare_dram_parameter("x0a", [P, EC * 256], F16, isOutput=False)
    x0b_d = nc.declare_dram_parameter("x0b", [P, EC * 256], F16, isOutput=False)
    x0f_d = nc.declare_dram_parameter("x0f", [P, EC * NB], F16, isOutput=False)
    x1_d = nc.declare_dram_parameter("x1", [P, EC * NB], F16, isOutput=False)
    x2_d = nc.declare_dram_parameter("x2", [P, EC * NB], F16, isOutput=False)
    x3_d = nc.declare_dram_parameter("x3", [P, EC * NB], F16, isOutput=False)
    wqT = nc.declare_dram_parameter("wqT", [P, EC * DQ], F16, isOutput=False)
    wkT = nc.declare_dram_parameter("wkT", [P, EC * HD], F16, isOutput=False)
    wvT = nc.declare_dram_parameter("wvT", [P, EC * HD], F16, isOutput=False)
    woT = nc.declare_dram_parameter("woT", [P, NHC * EMB], F16, isOutput=False)
    iden_d = nc.declare_dram_parameter("iden", [128, 128], F16, isOutput=False)
    ones_d = nc.declare_dram_parameter("ones", [128, 128], F16, isOutput=False)
    y = nc.declare_dram_parameter("y", [N, EMB], F16, isOutput=True)

    wqT_r = wqT[:].rearrange("p (c d) -> p c d", c=EC)   # (128, 16, 512)
    wkT_r = wkT[:].rearrange("p (c d) -> p c d", c=EC)   # (128, 16, 128)
    wvT_r = wvT[:].rearrange("p (c d) -> p c d", c=EC)
    woT_r = woT[:].rearrange("p (c e) -> p c e", c=NHC)  # (128, 4, 2048)
    x0a_r = x0a_d[:].rearrange("p (c n) -> p c n", c=EC)
    x0b_r = x0b_d[:].rearrange("p (c n) -> p c n", c=EC)
    x0f_r = x0f_d[:].rearrange("p (c n) -> p c n", c=EC)
    x1_r = x1_d[:].rearrange("p (c n) -> p c n", c=EC)
    x2_r = x2_d[:].rearrange("p (c n) -> p c n", c=EC)
    x3_r = x3_d[:].rearrange("p (c n) -> p c n", c=EC)

    with tile.TileContext(nc) as tc:
      with tc.tile_pool(name="consts", bufs=1) as consts, \
           tc.tile_pool(name="persist", bufs=1) as persist:
        identity = consts.tile([P, P], F16, tag="identity")
        allones = consts.tile([P, P], F16, tag="allones")
        xc0al = persist.tile([P, 8, 256], F16, tag="xc0al")
        xc0ah = persist.tile([P, 8, 256], F16, tag="xc0ah")
        xc0b = persist.tile([P, EC, 256], F16, tag="xc0b")
        xc0f = persist.tile([P, EC, NB], F16, tag="xc0f")
        xc1 = persist.tile([P, EC, NB], F16, tag="xc1")
        xc2 = persist.tile([P, EC, NB], F16, tag="xc2")
        xc3 = persist.tile([P, EC, NB], F16, tag="xc3")
        wkl = persist.tile([P, 8, HD], F16, tag="wkl")
        wkh = persist.tile([P, 8, HD], F16, tag="wkh")
        wv = persist.tile([P, EC, HD], F16, tag="wv")
        wq = persist.tile([P, EC, DQ], F16, tag="wq")
        wo = persist.tile([P, NHC, EMB], F16, tag="wo")

        # DMA issue is INTERLEAVED with the consuming compute in program
        # order: the tile scheduler coarsens DMA-completion waits to the
        # newest DMA issued before the consumer, so issuing only what is
        # needed next keeps the first matmuls from waiting on unrelated
        # transfers.  First two transfers (wk + x0a, 1.5MB) gate the start.
        nc.sync.dma_start(wkl[:], wkT_r[:, 0:8, :])
        nc.sync.dma_start(xc0al[:], x0a_r[:, 0:8, :])
        dma_rest = [
            (wkh[:], wkT_r[:, 8:16, :]),
            (xc0ah[:], x0a_r[:, 8:16, :]),
            (wv[:], wvT_r),
            (identity[:], iden_d[:]),
            (xc0b[:], x0b_r),
            (wq[:, :, 0:128], wqT_r[:, :, 0:128]),
            (xc1[:], x1_r),
            (xc2[:], x2_r),
            (xc3[:], x3_r),
            (wq[:, :, 128:320], wqT_r[:, :, 128:320]),
            (wq[:, :, 320:512], wqT_r[:, :, 320:512]),
            (allones[:], ones_d[:]),
            (xc0f[:], x0f_r),
            (wo[:, :, 0:1024], woT_r[:, :, 0:1024]),
            (wo[:, :, 1024:2048], woT_r[:, :, 1024:2048]),
        ]
        xp = [xc0f, xc1, xc2, xc3]   # 512-wide x tiles (qproj/cfill rhs)

        kT = persist.tile([P, N], F16, tag="kT")
        V = persist.tile([P, SC, HD], F16, tag="V")
        qT = [persist.tile([P, N], F16, tag=f"qT{g}", name=f"qT{g}")
              for g in range(NHC)]
        OT = [persist.tile([P, N], F16, tag=f"OT{g}", name=f"OT{g}")
              for g in range(NHC)]

        # -------- k/v projections with head-0 q proj interleaved --------
        # block 0 runs as two 256-wide sub-blocks on separate x tiles so
        # compute starts as soon as the first 1.5MB of DMA lands; after
        # each full 512 chunk, head 0's q projection for that chunk runs
        # (re-reads resident x, matching compute rate to DMA bandwidth)
        with tc.tile_pool(name="vTp", bufs=1) as vTp:
          vT = vTp.tile([P, N], F16, tag="vT")
          with tc.tile_pool(name="psA", bufs=3, space="PSUM") as psA, \
               tc.tile_pool(name="psA2", bufs=2, space="PSUM") as psA2, \
               tc.tile_pool(name="psT", bufs=1, space="PSUM") as psT, \
               tc.tile_pool(name="psQa", bufs=2, space="PSUM") as psQa:
            def dq(n):
                for _ in range(n):
                    if dma_rest:
                        nc.sync.dma_start(*dma_rest.pop(0))

            # issue schedule: (block_index, after_t) -> #dmas to issue
            dma_sched = {(0, 0): 2,   # wv, iden      (after k0a emitted)
                         (0, 1): 1,   # x0b           (after v0a)
                         (1, 0): 1,   # wq0           (after k0b)
                         (1, 1): 1,   # x1            (after v0b)
                         (2, 0): 1,   # x2            (after k1)
                         (2, 1): 1,   # x3            (after v1)
                         (3, 0): 2,   # wq mid+hi     (after k2)
                         (3, 1): 1,   # ones          (after v2)
                         (4, 0): 2,   # x0f, wo lo    (after k3)
                         (4, 1): 1}   # wo hi         (after v3)
            def wk_sl(e):
                return wkl[:, e, :] if e < 8 else wkh[:, e - 8, :]

            def x0a_sl(e):
                return xc0al[:, e, :] if e < 8 else xc0ah[:, e - 8, :]

            blocks = [(None, 0, 256), (xc0b, 256, 256),
                      (xc1, 512, NB), (xc2, 1024, NB), (xc3, 1536, NB)]
            for bi, (xt, base, bw) in enumerate(blocks):
                for t in range(2):
                    if bw == NB:
                        ps = psA.tile([P, NB], FP32, tag="psA",
                                      name=f"psKV_{base}_{t}")
                    else:
                        ps = psA2.tile([P, 256], FP32, tag="psA2",
                                       name=f"psKV_{base}_{t}")
                    for e in range(EC):
                        if bi == 0 and t == 0 and e == 8:
                            # second halves of wk/x0a issued mid-loop so the
                            # first 8 matmuls only gate on the first 0.75MB
                            dq(2)
                        w_ap = wk_sl(e) if t == 0 else wv[:, e, :]
                        x_ap = x0a_sl(e) if bi == 0 else xt[:, e, 0:bw]
                        nc.tensor.matmul(
                            ps[:], w_ap, x_ap,
                            start=(e == 0), stop=(e == EC - 1),
                        )
                    dq(dma_sched.get((bi, t), 0))
                    if t == 0:
                        nc.scalar.copy(kT[:, base:base + bw], ps[:])
                    else:
                        nc.scalar.copy(vT[:, base:base + bw], ps[:])
                # transpose the freshly-written vT s-chunks into V (PE)
                for j in range(base // P, (base + bw) // P):
                    pt = psT.tile([P, P], F16, tag="psT", name=f"psT_{j}")
                    nc.tensor.transpose(pt[:], vT[:, ts(j, P)], identity[:])
                    nc.scalar.copy(V[:, j, :], pt[:])
                if base + bw in (NB, 2 * NB, 3 * NB, 4 * NB):
                    # head-0 q projection for the completed 512-chunk;
                    # chunk 0 runs as two 256-wide pieces (split x tiles)
                    nb = (base + bw) // NB - 1
                    if nb == 0:
                        for piece in (0, 1):
                            qs = psQa.tile([P, 256], FP32, tag="psQa",
                                           name=f"psQ0_0_{piece}")
                            for e in range(EC):
                                xtp = x0a_sl(e) if piece == 0 \
                                    else xc0b[:, e, :]
                                nc.tensor.matmul(
                                    qs[:], wq[:, e, ts(0, HD)], xtp,
                                    start=(e == 0), stop=(e == EC - 1),
                                )
                            nc.vector.tensor_copy(
                                qT[0][:, 256 * piece:256 * piece + 256], qs[:])
                    else:
                        qs = psQa.tile([P, NB], FP32, tag="psQa",
                                       name=f"psQ0_{nb}")
                        for e in range(EC):
                            nc.tensor.matmul(
                                qs[:], wq[:, e, ts(0, HD)], xp[nb][:, e, :],
                                start=(e == 0), stop=(e == EC - 1),
                            )
                        nc.vector.tensor_copy(qT[0][:, ts(nb, NB)], qs[:])

          # ------------ attention with pipelined q projection ------------
          with tc.tile_pool(name="esp", bufs=3) as esp, \
               tc.tile_pool(name="lap", bufs=2) as lap, \
               tc.tile_pool(name="rbp", bufs=2) as rbp, \
               tc.tile_pool(name="yfp", bufs=4) as yfp, \
               tc.tile_pool(name="psS", bufs=4, space="PSUM") as psS, \
               tc.tile_pool(name="psO", bufs=2, space="PSUM") as psO, \
               tc.tile_pool(name="psQ", bufs=2, space="PSUM") as psQ:

            def qproj_step(g, jj, qbox):
                """One j-slot of head g's q projection (jj in 0..63).

                Steps two n-blocks per e-chunk so consecutive matmuls
                share the stationary wq slice (one weight load per e)."""
                p, s = divmod(jj, 32)
                e, half = divmod(s, 2)
                nb = 2 * p + half
                if s < 2:
                    qbox[half] = psQ.tile([P, NB], FP32, tag="psQ",
                                          name=f"psQ_{g}_{nb}")
                nc.tensor.matmul(
                    qbox[half][:], wq[:, e, ts(g, HD)], xp[nb][:, e, :],
                    start=(e == 0), stop=(e == EC - 1),
                )
                if s >= 30:
                    nc.vector.tensor_copy(qT[g][:, ts(nb, NB)], qbox[half][:])

            qbox = [None, None]

            # head 3 has no q projection to pipeline; fill its exp-wait
            # slots with the output projection for n-tiles 0..3 instead.
            # g-outer over ob pairs: each OT slice is loaded once per pair
            # (tiles from the otherwise-idle psQ bank; one accumulation
            # group per tile, never two groups in one bank)
            cst = {"k": 0, "t": [None, None], "y": [None, None]}

            def cfill_step(drain=False):
                k = cst["k"]
                if k >= 64:
                    return
                nt, r = divmod(k, 16)
                obp, rr = divmod(r, 8)
                gg, half = divmod(rr, 2)
                ob = 2 * obp + half
                if rr < 2:
                    cst["t"][half] = psQ.tile([P, NB], FP32, tag="psQ",
                                              name=f"cf_{nt}_{ob}")
                    cst["y"][half] = yfp.tile([P, NB], F16, tag="yf",
                                              name=f"cfy_{nt}_{ob}")
                nc.tensor.matmul(
                    cst["t"][half][:], OT[gg][:, ts(nt, P)],
                    wo[:, gg, ts(ob, NB)],
                    start=(gg == 0), stop=(gg == NHC - 1),
                )
                if rr >= 6:
                    # during attention the scalar engine is exp-saturated, so
                    # copies go to DVE; at drain time split the two copies
                    # across scalar+DVE so the psQ rotation is not paced by
                    # one engine
                    if drain and half == 0:
                        nc.scalar.copy(cst["y"][half][:], cst["t"][half][:])
                    else:
                        nc.vector.tensor_copy(cst["y"][half][:],
                                              cst["t"][half][:])
                    nc.sync.dma_start(y[ts(nt, P), ts(ob, NB)],
                                      cst["y"][half][:])
                cst["k"] += 1

            def finalize(g, m, lacc, ot_ps):
                # all-ones matmul: every partition gets the key-sum of
                # lacc -> reciprocal + normalize at full DVE width
                pool = psQ if g == NHC - 1 else psS
                psl = pool.tile([P, NB], FP32,
                                tag="psQ" if g == NHC - 1 else "psS",
                                name=f"psl_{g}_{m}")
                nc.tensor.matmul(psl[:], allones[:], lacc[:],
                                 start=True, stop=True)
                rb = rbp.tile([P, NB], FP32, tag="rb", name=f"rb_{g}_{m}")
                nc.vector.reciprocal_approx_fast(rb[:], psl[:])
                nc.vector.tensor_mul(OT[g][:, ts(m, NB)], ot_ps[:], rb[:])

            pending = None    # (g, m, lacc, ot_ps) of the previous quarter
            for g in range(NHC):
                for m in range(NQ):
                    msl = ts(m, NB)
                    lacc = lap.tile([P, NB], F16, tag="lacc",
                                    name=f"lacc_{g}_{m}")
                    ot_ps = psO.tile([P, NB], FP32, tag="psO",
                                     name=f"psO_{g}_{m}")
                    for j in range(SC):
                        s_ps = psS.tile([P, NB], FP32, tag="psS",
                                        name=f"psS_{g}_{m}_{j}")
                        nc.tensor.matmul(
                            s_ps[:], kT[:, ts(j, P)], qT[g][:, msl],
                            start=True, stop=True,
                        )
                        if g < NHC - 1:
                            qproj_step(g + 1, m * SC + j, qbox)
                        elif m > 1 or (m == 1 and j >= 4):
                            cfill_step()
                        es = esp.tile([P, NB], F16, tag="es",
                                      name=f"es_{g}_{m}_{j}")
                        nc.scalar.activation(
                            es[:], s_ps[:],
                            mybir.ActivationFunctionType.Exp,
                            scale=float(SCALE),
                        )
                        if j == 0:
                            nc.vector.tensor_copy(lacc[:], es[:])
                        else:
                            nc.vector.tensor_add(lacc[:], lacc[:], es[:])
                        nc.tensor.matmul(
                            ot_ps[:], V[:, j, :], es[:],
                            start=(j == 0), stop=(j == SC - 1),
                        )
                        if j == 1 and pending is not None:
                            finalize(*pending)   # overlap with this quarter
                            pending = None
                    pending = (g, m, lacc, ot_ps)
            finalize(*pending)

            # ------------- tail output projection (nt 4..15) -------------
            # stays inside the attention pool scope (no pool-close barrier):
            # four psS-bank accumulation groups per n-tile, g-outer so each
            # OT head slice is loaded once per tile; remaining cfill drain
            # steps interleave with the first tiles; copies alternate
            # scalar/DVE so psum rotation is not paced by one engine
            for nt in range(4, N // P):
                cfill_step(drain=True)
                cfill_step(drain=True)
                yps = [psS.tile([P, NB], FP32, tag="psS",
                                name=f"yp_{nt}_{ob}") for ob in range(4)]
                for g in range(NHC):
                    lhsT = OT[g][:, ts(nt, P)]
                    for ob in range(4):
                        nc.tensor.matmul(
                            yps[ob][:],
                            lhsT,
                            wo[:, g, ts(ob, NB)],
                            start=(g == 0), stop=(g == NHC - 1),
                        )
                for ob in range(4):
                    ysb = yfp.tile([P, NB], F16, tag="yf",
                                   name=f"ysb_{nt}_{ob}")
                    if ob % 2 == 0:
                        nc.scalar.copy(ysb[:], yps[ob][:])
                    else:
                        nc.vector.tensor_copy(ysb[:], yps[ob][:])
                    nc.sync.dma_start(y[ts(nt, P), ts(ob, NB)], ysb[:])
            while cst["k"] < 64:
                cfill_step(drain=True)

    nc.compile()
    return nc


def _r3(a2d, width):
    """(EMB, width) -> contiguous (P, EC*width) in (p, c, w) order."""
    return np.ascontiguousarray(
        a2d.reshape(EC, 128, width).transpose(1, 0, 2).reshape(128, -1))


def _in_maps(x, Wq, Wk, Wv, Wo):
    x = np.asarray(x, dtype=np.float32)
    Wq = np.asarray(Wq, dtype=np.float16)
    Wk = np.asarray(Wk, dtype=np.float16)
    Wv = np.asarray(Wv, dtype=np.float16)
    Wo = np.asarray(Wo, dtype=np.float16)
    iden = np.eye(128, dtype=np.float16)
    ones = np.ones((128, 128), dtype=np.float16)
    # per-batch x rearranged to (p, c, n) then chunked along n
    xrs = []
    for b in range(2):
        xr = x[b].T.astype(np.float16).reshape(EC, 128, N).transpose(1, 0, 2)
        xrs.append({
            "x0a": np.ascontiguousarray(xr[:, :, 0:256]).reshape(128, -1),
            "x0b": np.ascontiguousarray(xr[:, :, 256:512]).reshape(128, -1),
            "x0f": np.ascontiguousarray(xr[:, :, 0:512]).reshape(128, -1),
            "x1": np.ascontiguousarray(xr[:, :, 512:1024]).reshape(128, -1),
            "x2": np.ascontiguousarray(xr[:, :, 1024:1536]).reshape(128, -1),
            "x3": np.ascontiguousarray(xr[:, :, 1536:2048]).reshape(128, -1),
        })
    maps = []
    for core in range(8):
        b, h = divmod(core, 4)
        woT = np.ascontiguousarray(Wo[:, DQ * h:DQ * (h + 1)].T)  # (512, 2048)
        maps.append({
            **xrs[b],
            "wqT": _r3(np.ascontiguousarray(Wq[DQ * h:DQ * (h + 1), :].T), DQ),
            "wkT": _r3(np.ascontiguousarray(Wk[HD * h:HD * (h + 1), :].T), HD),
            "wvT": _r3(np.ascontiguousarray(Wv[HD * h:HD * (h + 1), :].T), HD),
            "woT": np.ascontiguousarray(
                woT.reshape(NHC, 128, EMB).transpose(1, 0, 2).reshape(128, -1)),
            "iden": iden,
            "ones": ones,
        })
    return maps


def run(x, Wq, Wk, Wv, Wo, **spmd_kwargs):
    """Build/compile (cached) and run; returns BassKernelResults."""
    global _NC
    if _NC is None:
        _NC = _build()
    from concourse.bass_utils import run_bass_kernel_spmd
    return run_bass_kernel_spmd(_NC, _in_maps(x, Wq, Wk, Wv, Wo),
                                list(range(8)), **spmd_kwargs)


def kernel(x, attn_mask=None, is_causal=None, Wq=None, Wk=None, Wv=None,
           Wo=None, **_ignored):
    res = run(x, Wq, Wk, Wv, Wo)
    y = np.zeros((2, N, EMB), dtype=np.float32)
    for core in range(8):
        y[core // 4] += res.results[core]["y"].astype(np.float32)
    return y


# revision 27
# speedup vs baseline: 1.0845x; 1.0845x over previous
"""GQA attention kernel for 8 trn2 NeuronCores.

Sharding: core = (b, h) with b = core//4 (batch), h = core%4 (kv head).
Each core handles q heads 4h..4h+3 (a contiguous 512-column block of Wq),
its own kv head (128 rows of Wk/Wv), and the matching 512-column slice of
Wo.  Per-core output is a partial y (row-parallel Wo); host sums the 4
fp16 partials per batch in fp32.

All matmuls run in fp16 (full-rate at 2.4 GHz) with fp32 PSUM
accumulation.  The attention j-loop is exp-bound on the scalar engine,
so the q projection of head g+1 is software-pipelined into head g's
attention loop (one projection matmul per j iteration) to keep the PE
busy during exp waits.  Softmax normalization: an all-ones [128,128]
matmul gives the partition-broadcast key sum in one PE op, followed by
a full-width DVE approx reciprocal and multiply.

DMA priority order feeds the pipeline, with block 0 of x split into two
separate 256-column tiles so the first k-projection never waits on
bytes it does not read (tile-granular DMA deps).  A redundant full
512-wide copy of block 0 (arriving late, off the critical path) serves
the pipelined q projections, which need 512-wide rhs slices.

Stationary-weight changes cost ~17ns/matmul on the PE queue, so loops
are ordered to reuse lhsT: q projections step two n-blocks per e-chunk
(one weight load feeds two matmuls), the in-attention output projection
(cfill) runs g-outer over ob pairs, and the tail output projection
accumulates all four 512-col blocks of one n-tile in a 4-bank PSUM
tile so each OT head slice is loaded once per tile.
"""

import numpy as np

EMB = 2048
N = 2048          # sequence length
HD = 128          # head dim
NHC = 4           # q heads per core
DQ = NHC * HD     # 512: per-core q concat dim
EC = 16           # e chunks of 128
SC = 16           # s chunks of 128
NB = 512          # n block size
NQ = 4            # n quarters in attention phase
SCALE = 1.0 / np.sqrt(HD)

_NC = None


def _build():
    import concourse.bass as bass
    from concourse import bacc
    import concourse.mybir as mybir
    import concourse.tile as tile
    from concourse.bass import ts

    FP32 = mybir.dt.float32
    F16 = mybir.dt.float16
    P = 128

    nc = bacc.Bacc("TRN2", target_bir_lowering=False, debug=False, num_devices=8)
    # all inputs host-prearranged to (partition, chunk, col) so every DMA
    # reads long contiguous runs (full-rate instead of 256-512B descriptors)
    x0a_d = nc.declare_dram_parameter("x0a", [P, EC * 256], F16, isOutput=False)
    x0b_d = nc.declare_dram_parameter("x0b", [P, EC * 256], F16, isOutput=False)
    x0f_d = nc.declare_dram_parameter("x0f", [P, EC * NB], F16, isOutput=False)
    x1_d = nc.declare_dram_parameter("x1", [P, EC * NB], F16, isOutput=False)
    x2_d = nc.declare_dram_parameter("x2", [P, EC * NB], F16, isOutput=False)
    x3_d = nc.declare_dram_parameter("x3", [P, EC * NB], F16, isOutput=False)
    wqT = nc.declare_dram_parameter("wqT", [P, EC * DQ], F16, isOutput=False)
    wkT = nc.declare_dram_parameter("wkT", [P, EC * HD], F16, isOutput=False)
    wvT = nc.declare_dram_parameter("wvT", [P, EC * HD], F16, isOutput=False)
    woT = nc.declare_dram_parameter("woT", [P, NHC * EMB], F16, isOutput=False)
    iden_d = nc.declare_dram_parameter("iden", [128, 128], F16, isOutput=False)
    ones_d = nc.declare_dram_parameter("ones", [128, 128], F16, isOutput=False)
    y = nc.declare_dram_parameter("y", [N, EMB], F16, isOutput=True)

    wqT_r = wqT[:].rearrange("p (c d) -> p c d", c=EC)   # (128, 16, 512)
    wkT_r = wkT[:].rearrange("p (c d) -> p c d", c=EC)   # (128, 16, 128)
    wvT_r = wvT[:].rearrange("p (c d) -> p c d", c=EC)
    woT_r = woT[:].rearrange("p (c e) -> p c e", c=NHC)  # (128, 4, 2048)
    x0a_r = x0a_d[:].rearrange("p (c n) -> p c n", c=EC)
    x0b_r = x0b_d[:].rearrange("p (c n) -> p c n", c=EC)
    x0f_r = x0f_d[:].rearrange("p (c n) -> p c n", c=EC)
    x1_r = x1_d[:].rearrange("p (c n) -> p c n", c=EC)
    x2_r = x2_d[:].rearrange("p (c n) -> p c n", c=EC)
    x3_r = x3_d[:].rearrange("p (c n) -> p c n", c=EC)

    with tile.TileContext(nc) as tc:
      with tc.tile_pool(name="consts", bufs=1) as consts, \
           tc.tile_pool(name="persist", bufs=1) as persist:
        identity = consts.tile([P, P], F16, tag="identity")
        allones = consts.tile([P, P], F16, tag="allones")
        xc0al = persist.tile([P, 8, 256], F16, tag="xc0al")
        xc0ah = persist.tile([P, 8, 256], F16, tag="xc0ah")
        xc0b = persist.tile([P, EC, 256], F16, tag="xc0b")
        xc0f = persist.tile([P, EC, NB], F16, tag="xc0f")
        xc1 = persist.tile([P, EC, NB], F16, tag="xc1")
        xc2 = persist.tile([P, EC, NB], F16, tag="xc2")
        xc3 = persist.tile([P, EC, NB], F16, tag="xc3")
        wkl = persist.tile([P, 8, HD], F16, tag="wkl")
        wkh = persist.tile([P, 8, HD], F16, tag="wkh")
        wv = persist.tile([P, EC, HD], F16, tag="wv")
        wq = persist.tile([P, EC, DQ], F16, tag="wq")
        wo = persist.tile([P, NHC, EMB], F16, tag="wo")

        # DMA issue is INTERLEAVED with the consuming compute in program
        # order: the tile scheduler coarsens DMA-completion waits to the
        # newest DMA issued before the consumer, so issuing only what is
        # needed next keeps the first matmuls from waiting on unrelated
        # transfers.  First two transfers (wk + x0a, 1.5MB) gate the start.
        nc.sync.dma_start(wkl[:], wkT_r[:, 0:8, :])
        nc.sync.dma_start(xc0al[:], x0a_r[:, 0:8, :])
        dma_rest = [
            (wkh[:], wkT_r[:, 8:16, :]),
            (xc0ah[:], x0a_r[:, 8:16, :]),
            (wv[:], wvT_r),
            (identity[:], iden_d[:]),
            (xc0b[:], x0b_r),
            (wq[:, :, 0:128], wqT_r[:, :, 0:128]),
            (xc1[:], x1_r),
            (xc2[:], x2_r),
            (xc3[:], x3_r),
            (wq[:, :, 128:320], wqT_r[:, :, 128:320]),
            (wq[:, :, 320:512], wqT_r[:, :, 320:512]),
            (allones[:], ones_d[:]),
            (xc0f[:], x0f_r),
            (wo[:, :, 0:1024], woT_r[:, :, 0:1024]),
            (wo[:, :, 1024:2048], woT_r[:, :, 1024:2048]),
        ]
        xp = [xc0f, xc1, xc2, xc3]   # 512-wide x tiles (qproj/cfill rhs)

        kT = persist.tile([P, N], F16, tag="kT")
        V = persist.tile([P, SC, HD], F16, tag="V")
        qT = [persist.tile([P, N], F16, tag=f"qT{g}", name=f"qT{g}")
              for g in range(NHC)]
        OT = [persist.tile([P, N], F16, tag=f"OT{g}", name=f"OT{g}")
              for g in range(NHC)]

        # -------- k/v projections with head-0 q proj interleaved --------
        # block 0 runs as two 256-wide sub-blocks on separate x tiles so
        # compute starts as soon as the first 1.5MB of DMA lands; after
        # each full 512 chunk, head 0's q projection for that chunk runs
        # (re-reads resident x, matching compute rate to DMA bandwidth)
        with tc.tile_pool(name="vTp", bufs=1) as vTp:
          vT = vTp.tile([P, N], F16, tag="vT")
          with tc.tile_pool(name="psA", bufs=3, space="PSUM") as psA, \
               tc.tile_pool(name="psA2", bufs=2, space="PSUM") as psA2, \
               tc.tile_pool(name="psT", bufs=1, space="PSUM") as psT, \
               tc.tile_pool(name="psQa", bufs=2, space="PSUM") as psQa:
            def dq(n):
                for _ in range(n):
                    if dma_rest:
                        nc.sync.dma_start(*dma_rest.pop(0))

            # issue schedule: (block_index, after_t) -> #dmas to issue
            dma_sched = {(0, 0): 2,   # wv, iden      (after k0a emitted)
                         (0, 1): 1,   # x0b           (after v0a)
                         (1, 0): 1,   # wq0           (after k0b)
                         (1, 1): 1,   # x1            (after v0b)
                         (2, 0): 1,   # x2            (after k1)
                         (2, 1): 1,   # x3            (after v1)
                         (3, 0): 2,   # wq mid+hi     (after k2)
                         (3, 1): 1,   # ones          (after v2)
                         (4, 0): 2,   # x0f, wo lo    (after k3)
                         (4, 1): 1}   # wo hi         (after v3)
            def wk_sl(e):
                return wkl[:, e, :] if e < 8 else wkh[:, e - 8, :]

            def x0a_sl(e):
                return xc0al[:, e, :] if e < 8 else xc0ah[:, e - 8, :]

            blocks = [(None, 0, 256), (xc0b, 256, 256),
                      (xc1, 512, NB), (xc2, 1024, NB), (xc3, 1536, NB)]
            for bi, (xt, base, bw) in enumerate(blocks):
                for t in range(2):
                    if bw == NB:
                        ps = psA.tile([P, NB], FP32, tag="psA",
                                      name=f"psKV_{base}_{t}")
                    else:
                        ps = psA2.tile([P, 256], FP32, tag="psA2",
                                       name=f"psKV_{base}_{t}")
                    for e in range(EC):
                        if bi == 0 and t == 0 and e == 8:
                            # second halves of wk/x0a issued mid-loop so the
                            # first 8 matmuls only gate on the first 0.75MB
                            dq(2)
                        w_ap = wk_sl(e) if t == 0 else wv[:, e, :]
                        x_ap = x0a_sl(e) if bi == 0 else xt[:, e, 0:bw]
                        nc.tensor.matmul(
                            ps[:], w_ap, x_ap,
                            start=(e == 0), stop=(e == EC - 1),
                        )
                    dq(dma_sched.get((bi, t), 0))
                    if t == 0:
                        nc.scalar.copy(kT[:, base:base + bw], ps[:])
                    else:
                        nc.scalar.copy(vT[:, base:base + bw], ps[:])
                # transpose the freshly-written vT s-chunks into V (PE)
                for j in range(base // P, (base + bw) // P):
                    pt = psT.tile([P, P], F16, tag="psT", name=f"psT_{j}")
                    nc.tensor.transpose(pt[:], vT[:, ts(j, P)], identity[:])
                    nc.scalar.copy(V[:, j, :], pt[:])
                if base + bw in (NB, 2 * NB, 3 * NB, 4 * NB):
                    # head-0 q projection for the completed 512-chunk;
                    # chunk 0 runs as two 256-wide pieces (split x tiles)
                    nb = (base + bw) // NB - 1
                    if nb == 0:
                        for piece in (0, 1):
                            qs = psQa.tile([P, 256], FP32, tag="psQa",
                                           name=f"psQ0_0_{piece}")
                            for e in range(EC):
                                xtp = x0a_sl(e) if piece == 0 \
                                    else xc0b[:, e, :]
                                nc.tensor.matmul(
                                    qs[:], wq[:, e, ts(0, HD)], xtp,
                                    start=(e == 0), stop=(e == EC - 1),
                                )
                            nc.vector.tensor_copy(
                                qT[0][:, 256 * piece:256 * piece + 256], qs[:])
                    else:
                        qs = psQa.tile([P, NB], FP32, tag="psQa",
                                       name=f"psQ0_{nb}")
                        for e in range(EC):
                            nc.tensor.matmul(
                                qs[:], wq[:, e, ts(0, HD)], xp[nb][:, e, :],
                                start=(e == 0), stop=(e == EC - 1),
                            )
                        nc.vector.tensor_copy(qT[0][:, ts(nb, NB)], qs[:])

          # ------------ attention with pipelined q projection ------------
          with tc.tile_pool(name="esp", bufs=3) as esp, \
               tc.tile_pool(name="lap", bufs=2) as lap, \
               tc.tile_pool(name="rbp", bufs=2) as rbp, \
               tc.tile_pool(name="yfp", bufs=4) as yfp, \
               tc.tile_pool(name="psS", bufs=4, space="PSUM") as psS, \
               tc.tile_pool(name="psO", bufs=2, space="PSUM") as psO, \
               tc.tile_pool(name="psQ", bufs=2, space="PSUM") as psQ:

            def qproj_step(g, jj, qbox):
                """One j-slot of head g's q projection (jj in 0..63).

                Steps two n-blocks per e-chunk so consecutive matmuls
                share the stationary wq slice (one weight load per e)."""
                p, s = divmod(jj, 32)
                e, half = divmod(s, 2)
                nb = 2 * p + half
                if s < 2:
                    qbox[half] = psQ.tile([P, NB], FP32, tag="psQ",
                                          name=f"psQ_{g}_{nb}")
                nc.tensor.matmul(
                    qbox[half][:], wq[:, e, ts(g, HD)], xp[nb][:, e, :],
                    start=(e == 0), stop=(e == EC - 1),
                )
                if s >= 30:
                    nc.vector.tensor_copy(qT[g][:, ts(nb, NB)], qbox[half][:])

            qbox = [None, None]

            # head 3 has no q projection to pipeline; fill its exp-wait
            # slots with the output projection for n-tiles 0..3 instead.
            # g-outer over ob pairs: each OT slice is loaded once per pair
            # (tiles from the otherwise-idle psQ bank; one accumulation
            # group per tile, never two groups in one bank)
            cst = {"k": 0, "t": [None, None], "y": [None, None]}

            def cfill_step(drain=False):
                k = cst["k"]
                if k >= 64:
                    return
                nt, r = divmod(k, 16)
                obp, rr = divmod(r, 8)
                gg, half = divmod(rr, 2)
                ob = 2 * obp + half
                if rr < 2:
                    cst["t"][half] = psQ.tile([P, NB], FP32, tag="psQ",
                                              name=f"cf_{nt}_{ob}")
                    cst["y"][half] = yfp.tile([P, NB], F16, tag="yf",
                                              name=f"cfy_{nt}_{ob}")
                nc.tensor.matmul(
                    cst["t"][half][:], OT[gg][:, ts(nt, P)],
                    wo[:, gg, ts(ob, NB)],
                    start=(gg == 0), stop=(gg == NHC - 1),
                )
                if rr >= 6:
                    # during attention the scalar engine is exp-saturated, so
                    # copies go to DVE; at drain time split the two copies
                    # across scalar+DVE so the psQ rotation is not paced by
                    # one engine
                    if drain and half == 0:
                        nc.scalar.copy(cst["y"][half][:], cst["t"][half][:])
                    else:
                        nc.vector.tensor_copy(cst["y"][half][:],
                                              cst["t"][half][:])
                    nc.sync.dma_start(y[ts(nt, P), ts(ob, NB)],
                                      cst["y"][half][:])
                cst["k"] += 1

            def finalize(g, m, lacc, ot_ps):
                # all-ones matmul: every partition gets the key-sum of
                # lacc -> reciprocal + normalize at full DVE width
                pool = psQ if g == NHC - 1 else psS
                psl = pool.tile([P, NB], FP32,
                                tag="psQ" if g == NHC - 1 else "psS",
                                name=f"psl_{g}_{m}")
                nc.tensor.matmul(psl[:], allones[:], lacc[:],
                                 start=True, stop=True)
                rb = rbp.tile([P, NB], FP32, tag="rb", name=f"rb_{g}_{m}")
                nc.vector.reciprocal_approx_fast(rb[:], psl[:])
                nc.vector.tensor_mul(OT[g][:, ts(m, NB)], ot_ps[:], rb[:])

            pending = None    # (g, m, lacc, ot_ps) of the previous quarter
            for g in range(NHC):
                for m in range(NQ):
                    msl = ts(m, NB)
                    lacc = lap.tile([P, NB], F16, tag="lacc",
                                    name=f"lacc_{g}_{m}")
                    ot_ps = psO.tile([P, NB], FP32, tag="psO",
                                     name=f"psO_{g}_{m}")
                    for j in range(SC):
                        s_ps = psS.tile([P, NB], FP32, tag="psS",
                                        name=f"psS_{g}_{m}_{j}")
                        nc.tensor.matmul(
                            s_ps[:], kT[:, ts(j, P)], qT[g][:, msl],
                            start=True, stop=True,
                        )
                        if g < NHC - 1:
                            qproj_step(g + 1, m * SC + j, qbox)
                        elif m > 1 or (m == 1 and j >= 4):
                            cfill_step()
                        es = esp.tile([P, NB], F16, tag="es",
                                      name=f"es_{g}_{m}_{j}")
                        nc.scalar.activation(
                            es[:], s_ps[:],
                            mybir.ActivationFunctionType.Exp,
                            scale=float(SCALE),
                        )
                        if j == 0:
                            nc.vector.tensor_copy(lacc[:], es[:])
                        else:
                            nc.vector.tensor_add(lacc[:], lacc[:], es[:])
                        nc.tensor.matmul(
                            ot_ps[:], V[:, j, :], es[:],
                            start=(j == 0), stop=(j == SC - 1),
                        )
                        if j == 1 and pending is not None:
                            finalize(*pending)   # overlap with this quarter
                            pending = None
                    pending = (g, m, lacc, ot_ps)
            finalize(*pending)

            # ------------- tail output projection (nt 4..15) -------------
            # stays inside the attention pool scope (no pool-close barrier):
            # four psS-bank accumulation groups per n-tile, g-outer so each
            # OT head slice is loaded once per tile; remaining cfill drain
            # steps interleave with the first tiles; copies alternate
            # scalar/DVE so psum rotation is not paced by one engine
            for nt in range(4, N // P):
                cfill_step(drain=True)
                cfill_step(drain=True)
                yps = [psS.tile([P, NB], FP32, tag="psS",
                                name=f"yp_{nt}_{ob}") for ob in range(4)]
                for g in range(NHC):
                    lhsT = OT[g][:, ts(nt, P)]
                    for ob in range(4):
                        nc.tensor.matmul(
                            yps[ob][:],
                            lhsT,
                            wo[:, g, ts(ob, NB)],
                            start=(g == 0), stop=(g == NHC - 1),
                        )
                for ob in range(4):
                    ysb = yfp.tile([P, NB], F16, tag="yf",
                                   name=f"ysb_{nt}_{ob}")
                    if ob % 2 == 0:
                        nc.scalar.copy(ysb[:], yps[ob][:])
                    else:
                        nc.vector.tensor_copy(ysb[:], yps[ob][:])
                    nc.sync.dma_start(y[ts(nt, P), ts(ob, NB)], ysb[:])
            while cst["k"] < 64:
                cfill_step(drain=True)

    nc.compile()
    return nc


def _r3(a2d, width):
    """(EMB, width) -> contiguous (P, EC*width) in (p, c, w) order."""
    return np.ascontiguousarray(
        a2d.reshape(EC, 128, width).transpose(1, 0, 2).reshape(128, -1))


def _in_maps(x, Wq, Wk, Wv, Wo):
    x = np.asarray(x, dtype=np.float32)
    Wq = np.asarray(Wq, dtype=np.float16)
    Wk = np.asarray(Wk, dtype=np.float16)
    Wv = np.asarray(Wv, dtype=np.float16)
    Wo = np.asarray(Wo, dtype=np.float16)
    iden = np.eye(128, dtype=np.float16)
    ones = np.ones((128, 128), dtype=np.float16)
    # per-batch x rearranged to (p, c, n) then chunked along n
    xrs = []
    for b in range(2):
        xr = x[b].T.astype(np.float16).reshape(EC, 128, N).transpose(1, 0, 2)
        xrs.append({
            "x0a": np.ascontiguousarray(xr[:, :, 0:256]).reshape(128, -1),
            "x0b": np.ascontiguousarray(xr[:, :, 256:512]).reshape(128, -1),
            "x0f": np.ascontiguousarray(xr[:, :, 0:512]).reshape(128, -1),
            "x1": np.ascontiguousarray(xr[:, :, 512:1024]).reshape(128, -1),
            "x2": np.ascontiguousarray(xr[:, :, 1024:1536]).reshape(128, -1),
            "x3": np.ascontiguousarray(xr[:, :, 1536:2048]).reshape(128, -1),
        })
    maps = []
    for core in range(8):
        b, h = divmod(core, 4)
        woT = np.ascontiguousarray(Wo[:, DQ * h:DQ * (h + 1)].T)  # (512, 2048)
        maps.append({
            **xrs[b],
            "wqT": _r3(np.ascontiguousarray(Wq[DQ * h:DQ * (h + 1), :].T), DQ),
            "wkT": _r3(np.ascontiguousarray(Wk[HD * h:HD * (h + 1), :].T), HD),
            "wvT": _r3(np.ascontiguousarray(Wv[HD * h:HD * (h + 1), :].T), HD),
            "woT": np.ascontiguousarray(
                woT.reshape(NHC, 128, EMB).transpose(1, 0, 2).reshape(128, -1)),
            "iden": iden,
            "ones": ones,
        })
    return maps


def run(x, Wq, Wk, Wv, Wo, **spmd_kwargs):
    """Build/compile (cached) and run; returns BassKernelResults."""
    global _NC
    if _NC is None:
        _NC = _build()
    from concourse.bass_utils import run_bass_kernel_spmd
    return run_bass_kernel_spmd(_NC, _in_maps(x, Wq, Wk, Wv, Wo),
                                list(range(8)), **spmd_kwargs)


def kernel(x, attn_mask=None, is_causal=None, Wq=None, Wk=None, Wv=None,
           Wo=None, **_ignored):
    res = run(x, Wq, Wk, Wv, Wo)
    y = np.zeros((2, N, EMB), dtype=np.float32)
    for core in range(8):
        y[core // 4] += res.results[core]["y"].astype(np.float32)
    return y
